# revision 17
# baseline (speedup 1.0000x reference)
"""Fused QKV-projection + attention-softmax kernel for Trainium2 (8 NeuronCores).

Computes softmax((X @ Wq)(X @ Wk)^T / sqrt(dkv)) == the reference nn_Attention
attn_weights output [B=2, H=16, L=2048, L=2048] fp32.

Sharding: data-parallel over batch x tensor-parallel over heads.
core i -> batch i//4, heads [4*(i%4) .. 4*(i%4)+4). Each core:
  1. loads X[b]^T (host pre-transposed, bf16) as XT [E, L] in SBUF
  2. projects Q^T/K^T per head pair in [feature, token] layout; the
     PSUM->SBUF copy (+bias) alternates ACT/DVE per 512-chunk
  3. scores = Q^T.T @ K^T per 128-query tile into PSUM (fp32)
  4. exp(s/8) is COLUMN-SPLIT across both elementwise engines per tile:
       ACT: hardware Exp on cols [0:SACT)
       DVE: cols [SACT:2048) via 2 chained custom ops (registered at
            import): op1 (8 ALU stages) computes z ~= e^(x/128) - 1
            from PSUM, op2 (5 stages) computes (1+z)^16 = e^(x*0.125)
            from SBUF.  The PSUM slot frees after max(ACT, op1), so the
            two score PSUM slots sustain ~1.5us/tile.
  5. unnormalized exp (bf16) DMAs to HBM in 1 MiB pairs, round-robin
     over three DMA queues (sync/scalar HWDGE + gpsimd SWDGE) to sustain
     the ~358 GB/s per-core HBM write limit; the host divides by the
     row sums during its bf16 -> fp32 upcast of the gathered output.
Projection pair 1 is emitted chunk-wise between the first tiles of head
1 so its PE work and PSUM residency overlap running exp instead of
stalling both exp engines for ~28us.
The V projection is dead code in the reference output and is skipped.
"""

from contextlib import ExitStack

import numpy as np

import concourse.bacc as bacc
import concourse.mybir as mybir
import concourse.tile as tile
from concourse.bass import ts
from concourse.bass_utils import run_bass_kernel_spmd

# ---------------------------------------------------------------------------
# Custom DVE exp ops, registered into concourse.dve_ops at import time (the
# documented extension point is editing dve_ops.py; the repo is read-only
# here so the identical registration is done programmatically).
# ---------------------------------------------------------------------------
import concourse.dve_ops as _dve_ops
from concourse.dve_spec import (
    C0,
    C1,
    C2,
    C3,
    One,
    Spec,
    Src0,
    _has_src1,
    _spill_c3_to_src1,
    lower as _dve_lower,
)
from concourse.dve_uop import DveOpSpec as _DveOpSpec

# z(v) = v + v^2*(g1 + g2 v + g3 v^2) ~= e^v - 1 on |v| <= 0.625, so that
# (1+z)^16 ~= e^(16 v); with v = raw_score/128 this yields exp(raw*0.125).
_G1, _G2, _G3 = 0.5008468962324599, 0.16865008563483236, 0.03833805586347763


def _exp_specs():
    v = Src0 * C0
    t = v * C1 + C2
    t2 = t * v + C3  # C3 = g1, spilled to a [P,1] in1 read via the swap flop
    w = t2 * v       # v*(g1 + g2 v + g3 v^2)
    z = w * v + v    # 8 ALU stages total

    def _ref_z(in0, in1, s0, s1, imm2):
        vv = in0.astype(np.float32) * s0
        return (vv * vv * ((vv * s1 + imm2) * vv + in1) + vv).astype(np.float32)

    spec_z = Spec(body=_spill_c3_to_src1(z), reference=_ref_z)

    q = Src0 + One
    y = q * q
    y2 = y * y
    y3 = y2 * y2  # body = y3*y3: (1+z)^16, 5 ALU stages

    def _ref_fin(in0, in1, s0, s1, imm2):
        qq = in0.astype(np.float32) + 1.0
        yy = qq * qq
        yy = yy * yy
        yy = yy * yy
        return (yy * yy).astype(np.float32)

    spec_fin = Spec(body=y3 * y3, reference=_ref_fin)
    return spec_z, spec_fin


def _register_dve_op(name, spec):
    for op in _dve_ops.OPS:
        if op.name == name:
            return op
    row = max(_dve_ops._SUB_OPCODE_FOR_NAME.values()) + 1
    assert row < 0x20, "custom-DVE row field overflow"
    _dve_ops._SUB_OPCODE_FOR_NAME[name] = row
    shas = {}
    for ver in ("v3", "v4"):
        try:
            uops = _dve_lower(spec, ver=ver)
            shas[ver] = _DveOpSpec(
                name=name, opcode=row, uops=uops, rd1_en=_has_src1(spec)
            ).sha(ver)
        except Exception:
            pass
    op = _dve_ops.DveOp(name, spec, subdim=False, uops_sha=shas)
    _dve_ops.OPS.append(op)
    _dve_ops.CUSTOM_DVE_SPECS[name] = spec
    return op


_SPEC_Z, _SPEC_FIN = _exp_specs()
EXP_Z = _register_dve_op("ANT_EXPZ16", _SPEC_Z)
EXP_FIN = _register_dve_op("ANT_EXPFIN16", _SPEC_FIN)

# ---------------------------------------------------------------------------

B, L, E = 2, 2048, 1024
H, DKV = 16, 64
HPC = 4          # heads per core
N_CORES = 8
P = 128
KT = E // P      # 8 contraction tiles for the projection
NQ = L // P      # 16 query tiles per head
NC512 = L // 512  # 4 512-wide chunks per row

F32 = mybir.dt.float32
BF16 = mybir.dt.bfloat16

MM_DT = BF16

# exp column split: ACT gets [0:SACT), DVE gets [SACT:L). Balances
# ACT (S+352)/1.2ns against DVE ((L-S+120)+(L-S+58))/0.96ns per tile.
SACT = 1440

# dummy matmuls emitted into each tile's c3 PSUM region before the real
# chunk matmuls (the real c3, start=True, overwrites the junk). They only
# wait on the DVE's early-finishing column read of the previous tile in the
# slot, so they run during what would otherwise be PE idle — keeping the
# PE activity monitor busy so HAM holds the 2.4 GHz grant through the
# exp-bound steady state (LDWEIGHTS alone does not count as activity).
MM_FILL = 3

# set by test.py to enable NTFF tracing; harness leaves it False
TRACE = False

_cached_nc = None


def _emit(tc, ctx):
    nc = tc.nc

    x_d = nc.dram_tensor("x", [E, L], MM_DT, kind="ExternalInput")  # X^T
    w_d = nc.dram_tensor("w", [E, HPC * P], MM_DT, kind="ExternalInput")
    b_d = nc.dram_tensor("bqk", [P, HPC], F32, kind="ExternalInput")
    out_d = nc.dram_tensor("out", [HPC, L, L], BF16, kind="ExternalOutput")

    const = ctx.enter_context(tc.tile_pool(name="const", bufs=1))
    xtp = ctx.enter_context(tc.tile_pool(name="xt", bufs=1))
    qkp = ctx.enter_context(tc.tile_pool(name="qk", bufs=2))
    expp = ctx.enter_context(tc.tile_pool(name="exp", bufs=5))
    zp = ctx.enter_context(tc.tile_pool(name="zp", bufs=3))
    smalls = ctx.enter_context(tc.tile_pool(name="smalls", bufs=4))

    psum = ctx.enter_context(tc.tile_pool(name="psum", bufs=1, space="PSUM"))

    # memsets first on the idle DVE so the PE warm-up and exp-table preload
    # are not gated on the DMA-busy queues
    warm = const.tile([P, 512], MM_DT, tag="warm")
    nc.vector.memset(warm[:], 0.0)
    # [P,1] fp32 broadcast constant for the DVE exp (g1 rides in1/Src1)
    g1c = const.tile([P, 1], F32, tag="g1c")
    nc.vector.memset(g1c[:], _G1)

    # W in kt-pair chunks so the first projection matmuls are gated on a
    # 0.25 MB transfer, not the whole 1 MB; xt chunks spread over three DMA
    # queues so ~5 MB of input lands in parallel.
    w_sb = const.tile([P, KT, HPC * P], MM_DT, tag="w")
    w_r = w_d[:].rearrange("(kt p) f -> p kt f", p=P)
    for wc in range(0, KT, 2):
        nc.sync.dma_start(w_sb[:, wc : wc + 2, :], w_r[:, wc : wc + 2, :])
    bias_sb = const.tile([P, HPC], F32, tag="bias")
    nc.gpsimd.dma_start(bias_sb[:], b_d[:])

    xt = xtp.tile([P, KT, L], MM_DT, tag="xt")
    xt_eng = {0: nc.scalar, 3: nc.scalar, 6: nc.scalar,
              1: nc.gpsimd, 4: nc.gpsimd, 7: nc.gpsimd,
              2: nc.sync, 5: nc.sync}
    for et in range(KT):
        xt_eng[et].dma_start(
            xt[:, et, :],
            x_d[ts(et, P), :],
        )

    # preload the exp table set on ACT during the DMA wait (~2.7us, once);
    # the later Identity bias-copies live in the same set.
    pre = smalls.tile([P, 1], F32, tag="pre")
    nc.scalar.activation(pre[:], warm[:, 0:1], mybir.ActivationFunctionType.Exp)

    # PE warm-up: dummy matmuls with no input deps keep the PE busy while
    # the first DMAs land, so HAM unthrottles (1.2 -> 2.4 GHz) before the
    # real projection starts.
    for _ in range(12):
        pw = psum.tile([P, 512], F32, tag="scores", bufs=2)
        nc.tensor.matmul(pw[:], warm[:, 0:P], warm[:], start=True, stop=True)

    def proj_copy(dst, pp, c, blk):
        b_ap = bias_sb[:, blk : blk + 1]
        if c % 2 == 0:
            nc.scalar.activation(
                dst[:, ts(c, 512)],
                pp[:, ts(c, 512)],
                mybir.ActivationFunctionType.Identity,
                bias=b_ap,
            )
        else:
            nc.vector.tensor_scalar_add(dst[:, ts(c, 512)], pp[:, ts(c, 512)], b_ap)

    # w columns are host-reordered: block 2*pair   = [Q_h0 | Q_h1] (128 feats)
    #                               block 2*pair+1 = [K_h0 | K_h1]
    def proj_part(dst, blk, c0, fill=0):
        """Two 512-col chunks of one projection destination (16 matmuls +
        2 copies) in their own PSUM allocation, so a part can interleave
        with a single scores tile without breaking the 2-slot rotation."""
        pp = psum.tile([P, L], F32, tag="scores", bufs=2)
        for c in (c0, c0 + 1):
            for k in range(KT):
                if fill and c == c0:
                    # keep HAM busy through early xt-arrival stalls; junk
                    # lands in the last chunk region, overwritten by its
                    # k==0 start=True matmul
                    nc.tensor.matmul(
                        pp[:, ts(3, 512)], warm[:, 0:P], warm[:],
                        start=True, stop=True,
                    )
                nc.tensor.matmul(
                    pp[:, ts(c, 512)],
                    w_sb[:, k, ts(blk, P)],
                    xt[:, k, ts(c, 512)],
                    start=(k == 0),
                    stop=(k == KT - 1),
                )
            proj_copy(dst, pp, c, blk)

    def proj_dst(dst, blk, fill=0):
        proj_part(dst, blk, 0, fill)
        proj_part(dst, blk, 2, fill)

    def proj_alloc(pair):
        qt = qkp.tile([P, L], MM_DT, tag="qt")  # 0:64 = Q^T h0, 64:128 = Q^T h1
        kt_t = qkp.tile([P, L], MM_DT, tag="kt")
        return qt, kt_t

    out_q = (nc.sync, nc.scalar, nc.gpsimd)
    dma_i = [0]

    def scores_tile(qt, kt_t, h, off, q, ex, z2, fill=MM_FILL):
        ps = psum.tile([P, L], F32, tag="scores", bufs=2)
        for _ in range(fill):
            nc.tensor.matmul(
                ps[:, ts(3, 512)], warm[:, 0:P], warm[:], start=True, stop=True
            )
        # c3 first: its region (and the fillers') only waits on the DVE's
        # early column read of the previous tile in this PSUM slot
        for c in (3, 2, 1, 0):
            nc.tensor.matmul(
                ps[:, ts(c, 512)],
                qt[off : off + DKV, ts(q, P)],
                kt_t[off : off + DKV, ts(c, 512)],
                start=True,
                stop=True,
            )
        # unnormalized exp, column-split across ACT and DVE; host divides
        # by the row sums
        nc.scalar.activation(
            ex[:, q % 2, 0:SACT],
            ps[:, 0:SACT],
            mybir.ActivationFunctionType.Exp,
            scale=1.0 / np.sqrt(DKV),
        )
        if SACT < L:
            nc.vector._custom_dve(
                EXP_Z, out=z2[:, q % 2, :], in0=ps[:, SACT:L], in1=g1c[:],
                s0=0.125 / 16.0, s1=_G3, imm2=_G2,
            )
        if q % 2 == 1:
            if SACT < L:
                # one fused (1+z)^16 over both tiles of the pair
                nc.vector._custom_dve(EXP_FIN, out=ex[:, :, SACT:L], in0=z2[:])
            # 1 MiB store (two q-tiles), round-robin over 3 DMA queues
            dst = out_d[h, ts(q // 2, 2 * P), :].rearrange("(t p) l -> p t l", p=P)
            eng = out_q[dma_i[0] % len(out_q)]
            dma_i[0] += 1
            eng.dma_start(dst, ex[:])

    def scores_head(qt, kt_t, h, off, interleave=()):
        """interleave: dict q -> callable emitted right after tile q."""
        ex = z2 = None
        for q in range(NQ):
            if q % 2 == 0:
                ex = expp.tile([P, 2, L], BF16, tag="exp")
                z2 = zp.tile([P, 2, L - SACT], F32, tag="z")
            fill = 0 if q in interleave or (q - 1) in interleave else MM_FILL
            scores_tile(qt, kt_t, h, off, q, ex, z2, fill)
            if q in interleave:
                interleave[q]()

    # pair0 projection up front (exp engines idle anyway during the ramp;
    # fill=1 bridges early xt-arrival stalls); pair1 spread chunk-pair-wise
    # across head 1's early tiles so each held projection allocation shares
    # the 2-slot PSUM rotation with exactly one scores tile.
    qt0, kt0 = proj_alloc(0)
    proj_dst(kt0, 1, fill=1)
    proj_dst(qt0, 0, fill=1)
    scores_head(qt0, kt0, 0, 0)
    qt1, kt1 = proj_alloc(1)
    scores_head(
        qt0,
        kt0,
        1,
        DKV,
        interleave={
            0: lambda: proj_part(kt1, 3, 0),
            2: lambda: proj_part(kt1, 3, 2),
            4: lambda: proj_part(qt1, 2, 0),
            6: lambda: proj_part(qt1, 2, 2),
        },
    )
    scores_head(qt1, kt1, 2, 0)
    scores_head(qt1, kt1, 3, DKV)


def build():
    global _cached_nc
    if _cached_nc is not None:
        return _cached_nc
    nc = bacc.Bacc("TRN2", target_bir_lowering=False, debug=False)
    with tile.TileContext(nc) as tc, ExitStack() as ctx:
        _emit(tc, ctx)
    nc.compile()
    _cached_nc = nc
    return nc


def _shard_inputs(X, W_qkv, b_qkv):
    X = np.ascontiguousarray(np.asarray(X, dtype=np.float32))
    W = np.asarray(W_qkv, dtype=np.float32)
    bq = np.asarray(b_qkv, dtype=np.float32)
    in_maps = []
    for core in range(N_CORES):
        b = core // 4
        g = core % 4
        heads = list(range(g * HPC, (g + 1) * HPC))
        # per head h: W cols [h*3*DKV, h*3*DKV+DKV) = Q feats,
        #             [h*3*DKV+DKV, h*3*DKV+2*DKV) = K feats.
        # Reorder into per-pair stacked blocks: [Q_h0|Q_h1], [K_h0|K_h1], ...
        wq = [W[:, h * 3 * DKV : h * 3 * DKV + DKV] for h in heads]
        wk = [W[:, h * 3 * DKV + DKV : h * 3 * DKV + 2 * DKV] for h in heads]
        bqh = [bq[h * 3 * DKV : h * 3 * DKV + DKV] for h in heads]
        bkh = [bq[h * 3 * DKV + DKV : h * 3 * DKV + 2 * DKV] for h in heads]
        w_blocks, b_blocks = [], []
        for pair in range(HPC // 2):
            w_blocks += [wq[2 * pair], wq[2 * pair + 1]]
            w_blocks += [wk[2 * pair], wk[2 * pair + 1]]
            b_blocks += [np.concatenate([bqh[2 * pair], bqh[2 * pair + 1]])]
            b_blocks += [np.concatenate([bkh[2 * pair], bkh[2 * pair + 1]])]
        mm_np = mybir.dt.np(MM_DT)
        w_sel = np.concatenate(w_blocks, axis=1)
        b_sel = np.stack(b_blocks, axis=1)
        in_maps.append(
            {
                "x": np.ascontiguousarray(X[b].T).astype(mm_np),
                "w": np.ascontiguousarray(w_sel).astype(mm_np),
                "bqk": np.ascontiguousarray(b_sel),
            }
        )
    return in_maps


def kernel(X, W_qkv, b_qkv):
    nc = build()
    in_maps = _shard_inputs(X, W_qkv, b_qkv)
    res = run_bass_kernel_spmd(nc, in_maps, core_ids=list(range(N_CORES)), trace=TRACE)
    out = np.empty((B, H, L, L), dtype=np.float32)
    for core in range(N_CORES):
        b = core // 4
        g = core % 4
        chunk = res.results[core]["out"].astype(np.float32)
        chunk /= chunk.sum(axis=-1, keepdims=True)
        out[b, g * HPC : (g + 1) * HPC] = chunk
    kernel.last_results = res
    return out


# revision 21
# speedup vs baseline: 1.0006x; 1.0006x over previous
"""Fused QKV-projection + attention-softmax kernel for Trainium2 (8 NeuronCores).

Computes softmax((X @ Wq)(X @ Wk)^T / sqrt(dkv)) == the reference nn_Attention
attn_weights output [B=2, H=16, L=2048, L=2048] fp32.

Sharding: data-parallel over batch x tensor-parallel over heads.
core i -> batch i//4, heads [4*(i%4) .. 4*(i%4)+4). Each core:
  1. loads X[b]^T (host pre-transposed, bf16) as XT [E, L] in SBUF
  2. projects Q^T/K^T per head pair in [feature, token] layout; the
     PSUM->SBUF copy (+bias) alternates ACT/DVE per 512-chunk
  3. scores = Q^T.T @ K^T per 128-query tile into PSUM (fp32)
  4. exp(s/8) is COLUMN-SPLIT across both elementwise engines per tile:
       ACT: hardware Exp on cols [0:SACT)
       DVE: cols [SACT:2048) via 2 chained custom ops (registered at
            import): op1 (8 ALU stages) computes z ~= e^(x/128) - 1
            from PSUM, op2 (5 stages) computes (1+z)^16 = e^(x*0.125)
            from SBUF.  The PSUM slot frees after max(ACT, op1), so the
            two score PSUM slots sustain ~1.5us/tile.
  5. unnormalized exp (bf16) DMAs to HBM in 1 MiB pairs, round-robin
     over three DMA queues (sync/scalar HWDGE + gpsimd SWDGE) to sustain
     the ~358 GB/s per-core HBM write limit; the host divides by the
     row sums during its bf16 -> fp32 upcast of the gathered output.
Projection pair 1 is emitted chunk-wise between the first tiles of head
1 so its PE work and PSUM residency overlap running exp instead of
stalling both exp engines for ~28us.
The V projection is dead code in the reference output and is skipped.
"""

from contextlib import ExitStack

import numpy as np

import concourse.bacc as bacc
import concourse.mybir as mybir
import concourse.tile as tile
from concourse.bass import ts
from concourse.bass_utils import run_bass_kernel_spmd

# ---------------------------------------------------------------------------
# Custom DVE exp ops, registered into concourse.dve_ops at import time (the
# documented extension point is editing dve_ops.py; the repo is read-only
# here so the identical registration is done programmatically).
# ---------------------------------------------------------------------------
import concourse.dve_ops as _dve_ops
from concourse.dve_spec import (
    C0,
    C1,
    C2,
    C3,
    One,
    Spec,
    Src0,
    _has_src1,
    _spill_c3_to_src1,
    lower as _dve_lower,
)
from concourse.dve_uop import DveOpSpec as _DveOpSpec

# z(v) = v + v^2*(g1 + g2 v + g3 v^2) ~= e^v - 1 on |v| <= 0.625, so that
# (1+z)^16 ~= e^(16 v); with v = raw_score/128 this yields exp(raw*0.125).
_G1, _G2, _G3 = 0.5008468962324599, 0.16865008563483236, 0.03833805586347763


def _exp_specs():
    v = Src0 * C0
    t = v * C1 + C2
    t2 = t * v + C3  # C3 = g1, spilled to a [P,1] in1 read via the swap flop
    w = t2 * v       # v*(g1 + g2 v + g3 v^2)
    z = w * v + v    # 8 ALU stages total

    def _ref_z(in0, in1, s0, s1, imm2):
        vv = in0.astype(np.float32) * s0
        return (vv * vv * ((vv * s1 + imm2) * vv + in1) + vv).astype(np.float32)

    spec_z = Spec(body=_spill_c3_to_src1(z), reference=_ref_z)

    q = Src0 + One
    y = q * q
    y2 = y * y
    y3 = y2 * y2  # body = y3*y3: (1+z)^16, 5 ALU stages

    def _ref_fin(in0, in1, s0, s1, imm2):
        qq = in0.astype(np.float32) + 1.0
        yy = qq * qq
        yy = yy * yy
        yy = yy * yy
        return (yy * yy).astype(np.float32)

    spec_fin = Spec(body=y3 * y3, reference=_ref_fin)
    return spec_z, spec_fin


def _register_dve_op(name, spec):
    for op in _dve_ops.OPS:
        if op.name == name:
            return op
    row = max(_dve_ops._SUB_OPCODE_FOR_NAME.values()) + 1
    assert row < 0x20, "custom-DVE row field overflow"
    _dve_ops._SUB_OPCODE_FOR_NAME[name] = row
    shas = {}
    for ver in ("v3", "v4"):
        try:
            uops = _dve_lower(spec, ver=ver)
            shas[ver] = _DveOpSpec(
                name=name, opcode=row, uops=uops, rd1_en=_has_src1(spec)
            ).sha(ver)
        except Exception:
            pass
    op = _dve_ops.DveOp(name, spec, subdim=False, uops_sha=shas)
    _dve_ops.OPS.append(op)
    _dve_ops.CUSTOM_DVE_SPECS[name] = spec
    return op


_SPEC_Z, _SPEC_FIN = _exp_specs()
EXP_Z = _register_dve_op("ANT_EXPZ16", _SPEC_Z)
EXP_FIN = _register_dve_op("ANT_EXPFIN16", _SPEC_FIN)

# ---------------------------------------------------------------------------

B, L, E = 2, 2048, 1024
H, DKV = 16, 64
HPC = 4          # heads per core
N_CORES = 8
P = 128
KT = E // P      # 8 contraction tiles for the projection
NQ = L // P      # 16 query tiles per head
NC512 = L // 512  # 4 512-wide chunks per row

F32 = mybir.dt.float32
BF16 = mybir.dt.bfloat16

MM_DT = BF16

# exp column split: ACT gets [0:SACT), DVE gets [SACT:L). Balances
# ACT (S+352)/1.2ns against DVE ((L-S+120)+(L-S+58))/0.96ns per tile.
SACT = 1440

# dummy matmuls emitted into each tile's c3 PSUM region before the real
# chunk matmuls (the real c3, start=True, overwrites the junk). They only
# wait on the DVE's early-finishing column read of the previous tile in the
# slot, so they run during what would otherwise be PE idle — keeping the
# PE activity monitor busy so HAM holds the 2.4 GHz grant through the
# exp-bound steady state (LDWEIGHTS alone does not count as activity).
MM_FILL = 2

# set by test.py to enable NTFF tracing; harness leaves it False
TRACE = False

_cached_nc = None


def _emit(tc, ctx):
    nc = tc.nc

    x_d = nc.dram_tensor("x", [E, L], MM_DT, kind="ExternalInput")  # X^T
    w_d = nc.dram_tensor("w", [E, HPC * P], MM_DT, kind="ExternalInput")
    b_d = nc.dram_tensor("bqk", [P, HPC], F32, kind="ExternalInput")
    out_d = nc.dram_tensor("out", [HPC, L, L], BF16, kind="ExternalOutput")

    const = ctx.enter_context(tc.tile_pool(name="const", bufs=1))
    xtp = ctx.enter_context(tc.tile_pool(name="xt", bufs=1))
    qkp = ctx.enter_context(tc.tile_pool(name="qk", bufs=2))
    expp = ctx.enter_context(tc.tile_pool(name="exp", bufs=5))
    zp = ctx.enter_context(tc.tile_pool(name="zp", bufs=3))
    smalls = ctx.enter_context(tc.tile_pool(name="smalls", bufs=4))

    psum = ctx.enter_context(tc.tile_pool(name="psum", bufs=1, space="PSUM"))

    # memsets first on the idle DVE so the PE warm-up and exp-table preload
    # are not gated on the DMA-busy queues
    warm = const.tile([P, 512], MM_DT, tag="warm")
    nc.vector.memset(warm[:], 0.0)
    # [P,1] fp32 broadcast constant for the DVE exp (g1 rides in1/Src1)
    g1c = const.tile([P, 1], F32, tag="g1c")
    nc.vector.memset(g1c[:], _G1)

    # W in kt-pair chunks split over two queues so the first projection
    # matmuls are gated on a 0.25 MB transfer, not the whole 1 MB; xt chunks
    # spread over three DMA queues so ~5 MB of input lands in parallel.
    w_sb = const.tile([P, KT, HPC * P], MM_DT, tag="w")
    w_r = w_d[:].rearrange("(kt p) f -> p kt f", p=P)
    nc.sync.dma_start(w_sb[:, 0:2, :], w_r[:, 0:2, :])
    nc.scalar.dma_start(w_sb[:, 2:4, :], w_r[:, 2:4, :])
    nc.sync.dma_start(w_sb[:, 4:6, :], w_r[:, 4:6, :])
    nc.scalar.dma_start(w_sb[:, 6:8, :], w_r[:, 6:8, :])
    bias_sb = const.tile([P, HPC], F32, tag="bias")
    nc.gpsimd.dma_start(bias_sb[:], b_d[:])

    xt = xtp.tile([P, KT, L], MM_DT, tag="xt")
    xt_eng = {0: nc.gpsimd, 1: nc.sync, 2: nc.scalar,
              3: nc.gpsimd, 4: nc.sync, 5: nc.scalar,
              6: nc.gpsimd, 7: nc.sync}
    for et in range(KT):
        xt_eng[et].dma_start(
            xt[:, et, :],
            x_d[ts(et, P), :],
        )

    # preload the exp table set on ACT during the DMA wait (~2.7us, once);
    # the later Identity bias-copies live in the same set.
    pre = smalls.tile([P, 1], F32, tag="pre")
    nc.scalar.activation(pre[:], warm[:, 0:1], mybir.ActivationFunctionType.Exp)

    # PE warm-up: dummy matmuls with no input deps keep the PE busy while
    # the first DMAs land, so HAM unthrottles (1.2 -> 2.4 GHz) before the
    # real projection starts.
    for _ in range(12):
        pw = psum.tile([P, 512], F32, tag="scores", bufs=2)
        nc.tensor.matmul(pw[:], warm[:, 0:P], warm[:], start=True, stop=True)

    def proj_copy(dst, pp, c, blk):
        b_ap = bias_sb[:, blk : blk + 1]
        if c % 2 == 0:
            nc.scalar.activation(
                dst[:, ts(c, 512)],
                pp[:, ts(c, 512)],
                mybir.ActivationFunctionType.Identity,
                bias=b_ap,
            )
        else:
            nc.vector.tensor_scalar_add(dst[:, ts(c, 512)], pp[:, ts(c, 512)], b_ap)

    # w columns are host-reordered: block 2*pair   = [Q_h0 | Q_h1] (128 feats)
    #                               block 2*pair+1 = [K_h0 | K_h1]
    def proj_part(dst, blk, c0, nchunk=2):
        """`nchunk` 512-col chunks of one projection destination in their own
        PSUM allocation, k-OUTER: each arriving xt chunk feeds matmuls for
        every region immediately, so the part completes one sweep after the
        last xt chunk lands (and the stationary is reused across regions)."""
        pp = psum.tile([P, L], F32, tag="scores", bufs=2)
        for k in range(KT):
            for c in range(c0, c0 + nchunk):
                nc.tensor.matmul(
                    pp[:, ts(c, 512)],
                    w_sb[:, k, ts(blk, P)],
                    xt[:, k, ts(c, 512)],
                    start=(k == 0),
                    stop=(k == KT - 1),
                )
        for c in range(c0, c0 + nchunk):
            proj_copy(dst, pp, c, blk)

    def proj_dst(dst, blk):
        proj_part(dst, blk, 0, nchunk=NC512)

    def proj_alloc(pair):
        qt = qkp.tile([P, L], MM_DT, tag="qt")  # 0:64 = Q^T h0, 64:128 = Q^T h1
        kt_t = qkp.tile([P, L], MM_DT, tag="kt")
        return qt, kt_t

    out_q = (nc.sync, nc.scalar, nc.gpsimd)
    dma_i = [0]

    def scores_tile(qt, kt_t, h, off, q, ex, z2, fill=MM_FILL):
        ps = psum.tile([P, L], F32, tag="scores", bufs=2)
        # fillers + c3 live in the DVE's column region, released early by
        # op1 of the previous tile in this slot; c0 is emitted LAST so the
        # ACT of this tile launches as soon as possible after ACT(k-2)
        # releases cols [0:SACT) for c2/c1/c0.
        for _ in range(fill):
            nc.tensor.matmul(
                ps[:, ts(3, 512)], warm[:, 0:P], warm[:], start=True, stop=True
            )
        for c in (3, 2, 1, 0):
            nc.tensor.matmul(
                ps[:, ts(c, 512)],
                qt[off : off + DKV, ts(q, P)],
                kt_t[off : off + DKV, ts(c, 512)],
                start=True,
                stop=True,
            )
        # unnormalized exp, column-split across ACT and DVE; host divides
        # by the row sums
        nc.scalar.activation(
            ex[:, q % 2, 0:SACT],
            ps[:, 0:SACT],
            mybir.ActivationFunctionType.Exp,
            scale=1.0 / np.sqrt(DKV),
        )
        if SACT < L:
            nc.vector._custom_dve(
                EXP_Z, out=z2[:, q % 2, :], in0=ps[:, SACT:L], in1=g1c[:],
                s0=0.125 / 16.0, s1=_G3, imm2=_G2,
            )
        if q % 2 == 1:
            if SACT < L:
                # one fused (1+z)^16 over both tiles of the pair
                nc.vector._custom_dve(EXP_FIN, out=ex[:, :, SACT:L], in0=z2[:])
            # 1 MiB store (two q-tiles), round-robin over 3 DMA queues
            dst = out_d[h, ts(q // 2, 2 * P), :].rearrange("(t p) l -> p t l", p=P)
            eng = out_q[dma_i[0] % len(out_q)]
            dma_i[0] += 1
            eng.dma_start(dst, ex[:])

    def scores_head(qt, kt_t, h, off, interleave=()):
        """interleave: dict q -> callable emitted right after tile q."""
        ex = z2 = None
        for q in range(NQ):
            if q % 2 == 0:
                ex = expp.tile([P, 2, L], BF16, tag="exp")
                z2 = zp.tile([P, 2, L - SACT], F32, tag="z")
            fill = 0 if q in interleave or (q - 1) in interleave else MM_FILL
            scores_tile(qt, kt_t, h, off, q, ex, z2, fill)
            if q in interleave:
                interleave[q]()

    # pair0 projection up front (exp engines idle anyway during the ramp;
    # fill=1 bridges early xt-arrival stalls); pair1 spread chunk-pair-wise
    # across head 1's early tiles so each held projection allocation shares
    # the 2-slot PSUM rotation with exactly one scores tile.
    qt0, kt0 = proj_alloc(0)
    proj_dst(kt0, 1)
    proj_dst(qt0, 0)
    scores_head(qt0, kt0, 0, 0)
    qt1, kt1 = proj_alloc(1)
    scores_head(
        qt0,
        kt0,
        1,
        DKV,
        interleave={
            0: lambda: proj_part(kt1, 3, 0),
            2: lambda: proj_part(kt1, 3, 2),
            4: lambda: proj_part(qt1, 2, 0),
            6: lambda: proj_part(qt1, 2, 2),
        },
    )
    scores_head(qt1, kt1, 2, 0)
    scores_head(qt1, kt1, 3, DKV)


def build():
    global _cached_nc
    if _cached_nc is not None:
        return _cached_nc
    nc = bacc.Bacc("TRN2", target_bir_lowering=False, debug=False)
    with tile.TileContext(nc) as tc, ExitStack() as ctx:
        _emit(tc, ctx)
    nc.compile()
    _cached_nc = nc
    return nc


def _shard_inputs(X, W_qkv, b_qkv):
    X = np.ascontiguousarray(np.asarray(X, dtype=np.float32))
    W = np.asarray(W_qkv, dtype=np.float32)
    bq = np.asarray(b_qkv, dtype=np.float32)
    in_maps = []
    for core in range(N_CORES):
        b = core // 4
        g = core % 4
        heads = list(range(g * HPC, (g + 1) * HPC))
        # per head h: W cols [h*3*DKV, h*3*DKV+DKV) = Q feats,
        #             [h*3*DKV+DKV, h*3*DKV+2*DKV) = K feats.
        # Reorder into per-pair stacked blocks: [Q_h0|Q_h1], [K_h0|K_h1], ...
        wq = [W[:, h * 3 * DKV : h * 3 * DKV + DKV] for h in heads]
        wk = [W[:, h * 3 * DKV + DKV : h * 3 * DKV + 2 * DKV] for h in heads]
        bqh = [bq[h * 3 * DKV : h * 3 * DKV + DKV] for h in heads]
        bkh = [bq[h * 3 * DKV + DKV : h * 3 * DKV + 2 * DKV] for h in heads]
        w_blocks, b_blocks = [], []
        for pair in range(HPC // 2):
            w_blocks += [wq[2 * pair], wq[2 * pair + 1]]
            w_blocks += [wk[2 * pair], wk[2 * pair + 1]]
            b_blocks += [np.concatenate([bqh[2 * pair], bqh[2 * pair + 1]])]
            b_blocks += [np.concatenate([bkh[2 * pair], bkh[2 * pair + 1]])]
        mm_np = mybir.dt.np(MM_DT)
        w_sel = np.concatenate(w_blocks, axis=1)
        b_sel = np.stack(b_blocks, axis=1)
        in_maps.append(
            {
                "x": np.ascontiguousarray(X[b].T).astype(mm_np),
                "w": np.ascontiguousarray(w_sel).astype(mm_np),
                "bqk": np.ascontiguousarray(b_sel),
            }
        )
    return in_maps


def kernel(X, W_qkv, b_qkv):
    nc = build()
    in_maps = _shard_inputs(X, W_qkv, b_qkv)
    res = run_bass_kernel_spmd(nc, in_maps, core_ids=list(range(N_CORES)), trace=TRACE)
    out = np.empty((B, H, L, L), dtype=np.float32)
    for core in range(N_CORES):
        b = core // 4
        g = core % 4
        chunk = res.results[core]["out"].astype(np.float32)
        chunk /= chunk.sum(axis=-1, keepdims=True)
        out[b, g * HPC : (g + 1) * HPC] = chunk
    kernel.last_results = res
    return out
